# revision 4
# baseline (speedup 1.0000x reference)
"""Trainium2 Bass kernel for causal attention (scores = K @ Q^T variant), v3.

Problem (hardcoded):
  x  [8, 2048, 2048] f32, Wk/Wq/Wv [2048, 256] f32
  per batch b: K = x_b @ Wk, Q = x_b @ Wq, V = x_b @ Wv
  w = K @ Q^T / sqrt(256), causal-masked (strict upper = -inf),
  attn = softmax(w, axis=-1), out_b = attn @ V    -> [8, 2048, 256] f32

Sharding: data-parallel over batch, one batch element per NeuronCore (8 cores).

v3 design:
  - bf16 everywhere on the PE (streams at full clock, vs ~0.75x for f32r).
  - x^T via PE transposes in bf16 (1.0 cycles/row; the DMA XBAR alternative
    costs ~40+us of DMA queue time and stalls the pipeline).
  - scores computed TRANSPOSED (P^T[s,t] layout) so exp output feeds the
    attn @ V matmul directly as the stationary operand -- no PE transposes
    in stage 2. Softmax denominator comes from a ones-column in V.
  - ramped t-blocks [1,1,2,4,4,4] chunks so the PE starts on chunk 0.
"""
import sys

for _p in ("/opt/trn_rl_repo",):
    if _p not in sys.path:
        sys.path.insert(0, _p)

import numpy as np

import concourse.bass as bass  # noqa: F401  (registers AP machinery)
import concourse.mybir as mybir
from concourse import bacc
from concourse.tile import TileContext
from concourse.bass_utils import run_bass_kernel_spmd
from concourse.masks import make_identity

F32 = mybir.dt.float32
BF16 = mybir.dt.bfloat16

P = 128          # partitions
T = 2048         # sequence length (== E by construction of the module)
E = 2048         # embedding dim
D = 256          # head dim
EC = E // P      # 16 e-chunks
NT = T // P      # 16 t/s chunks of 128
DAV = D + 2      # V free dim incl ones column (256) + pad (257)
SCALE = 1.0 / 16.0   # 1/sqrt(D)
MASKVAL = -1e9

# chunk-ramped t-blocks: early blocks small so the PE starts ASAP
BLOCKS = [[0, 1], [2, 3], [4, 5, 6, 7], [8, 9, 10, 11], [12, 13, 14, 15]]

N_CORES = 8


def _build():
    nc = bacc.Bacc("TRN2", target_bir_lowering=False, debug=False,
                   num_devices=N_CORES)
    x_h = nc.dram_tensor("x", [T, E], F32, kind="ExternalInput")
    wk_h = nc.dram_tensor("Wk", [E, D], F32, kind="ExternalInput")
    wq_h = nc.dram_tensor("Wq", [E, D], F32, kind="ExternalInput")
    wv_h = nc.dram_tensor("Wv", [E, D], F32, kind="ExternalInput")
    y_h = nc.dram_tensor("out", [T, D], F32, kind="ExternalOutput")
    x_ap, y_ap = x_h.ap(), y_h.ap()

    with TileContext(nc) as tc:
        with tc.tile_pool(name="persist", bufs=1) as persist:
            # gpsimd-built constants FIRST so they don't queue behind the
            # SWDGE weight descriptor generation.
            ident_f = persist.tile([P, P], F32, name="ident_f")
            make_identity(nc, ident_f[:])
            ident_b = persist.tile([P, P], BF16, name="ident_b")
            nc.vector.tensor_copy(ident_b[:], ident_f[:])

            # transposed-causal 0/1 triangle mask [s=p, t=jj] for diagonal
            # tiles, applied multiplicatively AFTER exp (keeps the mask off
            # the score->exp critical path): 0 where jj < p (t < s), 1 else.
            trimask = persist.tile([P, P], BF16, name="trimask")
            nc.vector.memset(trimask[:], 1.0)
            nc.gpsimd.affine_select(
                out=trimask[:], in_=trimask[:],
                compare_op=mybir.AluOpType.is_ge, fill=0.0,
                base=0, pattern=[[1, P]], channel_multiplier=-1,
            )

            # --- persistent tensors -------------------------------------
            # e-dimension partition grouping is stride-16 (e = 16*p + pp):
            # weights then load CONTIGUOUSLY (16KB runs, full HBM bandwidth,
            # no gather) as [p, pp, d], and the x^T PE transposes consume
            # stride-16 column slices of x so both sides contract over the
            # same e-subsets. K^T/Q^T/V outputs are standard layout.
            # Weight delivery (SWDGE ring is descriptor-bound at ~100GB/s,
            # so only the later-consumed wq/wv ride it, in parallel with x):
            #  - wk: f32 staging DMA on the scalar HWDGE ring, slotted
            #    between x chunks c1 and c2 (FIFO within the ring), then a
            #    gpsimd cast (that engine is otherwise idle).
            #  - wq, wv: SWDGE cast DMA, contiguous pattern.
            wk_b = persist.tile([P, EC, D], BF16, name="wk_b")
            wq_b = persist.tile([P, EC, D], BF16, name="wq_b")
            wv_b = persist.tile([P, EC, D], BF16, name="wv_b")
            wk_f = persist.tile([P, EC, D], F32, name="wk_f")
            for wt, h in ((wv_b, wv_h), (wq_b, wq_h)):
                nc.gpsimd.dma_start(
                    wt[:], h.ap().rearrange("(p r) d -> p r d", p=P))

            kt = persist.tile([P, 2, T], BF16, name="kt")     # K^T [d, t]
            qt = persist.tile([P, 2, T], BF16, name="qt")     # Q^T [d, s]
            # V rows [s, d] plus ones column at d=256 (softmax denominator)
            v_sb = persist.tile([P, NT, DAV], BF16, name="v_sb")
            nc.vector.memset(v_sb[:, :, D:DAV], 0.0)
            nc.vector.memset(v_sb[:, :, D:D + 1], 1.0)

            # --- stage 1: load x, PE-transpose to x^T, project ----------
            with tc.tile_pool(name="s1", bufs=1) as s1, \
                 tc.tile_pool(name="s1ps", bufs=1, space="PSUM") as s1ps:

                xtbs = {}

                def load_chunk(bi, ci, c):
                    """x chunk c -> bf16 x^T into xtbs[bi][:, :, ci*P:...]."""
                    xtb = xtbs[bi]
                    x_b = s1.tile([P, E], BF16, name="x_b", tag="x_b", bufs=4)
                    x_f = s1.tile([P, E], F32, name="x_f", tag="x_f",
                                  bufs=4)
                    # sync ring: pure FIFO issuer (the scalar ring would
                    # serialize DMA issues behind the cast instructions)
                    nc.sync.dma_start(x_f[:],
                                      x_ap[c * P:(c + 1) * P, :])
                    # cast in halves so the first transposes start earlier
                    nc.scalar.copy(x_b[:, 0:E // 2], x_f[:, 0:E // 2])
                    nc.scalar.copy(x_b[:, E // 2:E], x_f[:, E // 2:E])
                    # stride-16 e-subset view matching the weight layout:
                    # x_br[:, pp, eh] = x_b[:, 16*eh + pp]
                    x_br = x_b[:].rearrange("a (b c) -> a c b", c=EC)
                    for ppg in range(EC // 4):
                        tr_ps = s1ps.tile([P, 4, P], BF16, name="tr_ps",
                                          tag="tr", bufs=4)
                        for j in range(4):
                            nc.tensor.transpose(
                                tr_ps[:, j], x_br[:, ppg * 4 + j, :],
                                ident_b[:])
                        nc.vector.tensor_copy(
                            xtb[:, ppg * 4:(ppg + 1) * 4, ci * P:(ci + 1) * P],
                            tr_ps[:])

                def proj_w(bi, wt, dst):
                    chunks = BLOCKS[bi]
                    ncch = len(chunks)
                    t0 = chunks[0] * P
                    xtb = xtbs[bi]
                    for dc in range(2):
                        pp = s1ps.tile([P, 512], F32, name="pp",
                                       tag="proj", bufs=4)
                        for ec in range(EC):
                            nc.tensor.matmul(
                                pp[:, 0:ncch * P],
                                wt[:, ec, dc * P:(dc + 1) * P],
                                xtb[:, ec, 0:ncch * P],
                                start=(ec == 0), stop=(ec == EC - 1))
                        nc.vector.tensor_copy(
                            dst[:, dc, t0:t0 + ncch * P],
                            pp[:, 0:ncch * P])

                def proj_v(bi, chunks):
                    xtb = xtbs[bi]
                    for ci, c in enumerate(chunks):
                        pv = s1ps.tile([P, D], F32, name="pv", tag="proj",
                                       bufs=4)
                        for ec in range(EC):
                            nc.tensor.matmul(
                                pv[:], xtb[:, ec, ci * P:(ci + 1) * P],
                                wv_b[:, ec, :],
                                start=(ec == 0), stop=(ec == EC - 1))
                        nc.vector.tensor_copy(v_sb[:, c, 0:D], pv[:])

                def loads(bi):
                    xtbs[bi] = s1.tile([P, EC, 512], BF16, name="xtb",
                                       tag="xtb", bufs=4)
                    for ci, c in enumerate(BLOCKS[bi]):
                        load_chunk(bi, ci, c)

                # software pipeline: transposes for block bi+2 are emitted
                # before KQ(bi+1) so the PE has transpose work to chew on
                # while the serial SWDGE weight stream trickles in; V is
                # deferred two blocks behind its (late-arriving) Wv.
                # PE schedule ordered by weight arrival: wv lands first on
                # the (otherwise idle) SWDGE ring, so V projections lead;
                # wk rides the x HWDGE ring behind chunks 0-3 (K second);
                # wq trails on SWDGE behind wv (Q third, by when the PE has
                # 40+us of queued T/V/K work anyway).
                loads(0)
                nc.sync.dma_start(
                    wk_f[:], wk_h.ap().rearrange("(p r) d -> p r d", p=P))
                loads(1)
                nc.vector.tensor_copy(wk_b[:], wk_f[:])
                proj_v(0, BLOCKS[0])
                proj_w(0, wk_b, kt)
                loads(2)
                proj_v(1, BLOCKS[1])
                proj_w(1, wk_b, kt)
                loads(3)
                proj_w(0, wq_b, qt)
                proj_w(1, wq_b, qt)
                proj_v(2, BLOCKS[2])
                proj_w(2, wk_b, kt)
                loads(4)
                proj_w(2, wq_b, qt)
                proj_v(3, BLOCKS[3])
                proj_w(3, wk_b, kt)
                proj_w(3, wq_b, qt)
                proj_v(4, BLOCKS[4])
                proj_w(4, wk_b, kt)
                proj_w(4, wq_b, qt)

            # --- stage 2: causal attention, transposed-P layout ---------
            # loop t-blocks of 512; for each, s-chunks from the diagonal
            # down to 0. P^T[s,t] tiles go straight into attn@V as lhsT.
            with tc.tile_pool(name="s2", bufs=1) as s2, \
                 tc.tile_pool(name="s2ps", bufs=1, space="PSUM") as s2ps:
                for tbi in range(4):
                    tb0 = tbi * 512
                    out_ps = [s2ps.tile([P, 512], F32, name=f"out_ps{q}",
                                        tag=f"out{q}", bufs=1)
                              for q in range(4)]
                    # ascending s-chunks: t-tile tq's accumulation stops at
                    # its diagonal (sc == 4*tbi+tq), so tiles retire one by
                    # one during the final 4 iterations instead of all at
                    # the end -- shorter tail.
                    for sc in range(0, 4 * tbi + 4):
                        q = sc - 4 * tbi           # >=0 on the diagonal
                        j0 = max(q, 0) * P         # first valid t col
                        w = 512 - j0
                        sc_ps = s2ps.tile([P, 512], F32, name="sc_ps",
                                          tag="sc", bufs=4)
                        for dc in range(2):
                            nc.tensor.matmul(
                                sc_ps[:, 0:w],
                                qt[:, dc, sc * P:(sc + 1) * P],
                                kt[:, dc, tb0 + j0:tb0 + 512],
                                start=(dc == 0), stop=(dc == 1))
                        p_sb = s2.tile([P, 512], BF16, name="p_sb", tag="p",
                                       bufs=4)
                        nc.scalar.activation(
                            p_sb[:, 0:w], sc_ps[:, 0:w],
                            mybir.ActivationFunctionType.Exp, scale=SCALE)
                        if q >= 0:
                            # zero the strict-lower triangle of the diagonal
                            # block (cols 0:128 of this tile)
                            nc.vector.tensor_mul(
                                p_sb[:, 0:P], p_sb[:, 0:P], trimask[:])
                        for tq in range(max(q, 0), 4):
                            nc.tensor.matmul(
                                out_ps[tq][:, 0:DAV],
                                p_sb[:, tq * P - j0:(tq + 1) * P - j0],
                                v_sb[:, sc, :],
                                start=(sc == 0), stop=(q == tq))
                        if q >= 0:
                            tt = 4 * tbi + q
                            rec = s2.tile([P, 1], F32, name="rec", tag="rec",
                                          bufs=2)
                            nc.vector.reciprocal(rec[:],
                                                 out_ps[q][:, D:D + 1])
                            o_sb = s2.tile([P, D], F32, name="o_sb",
                                           tag="osb", bufs=3)
                            nc.vector.tensor_scalar_mul(
                                o_sb[:], out_ps[q][:, 0:D], rec[:])
                            nc.sync.dma_start(y_ap[tt * P:(tt + 1) * P, :],
                                              o_sb[:])

    nc.compile()
    return nc


_NC_CACHE = None


def _get_nc():
    global _NC_CACHE
    if _NC_CACHE is None:
        _NC_CACHE = _build()
    return _NC_CACHE


def run(inputs: dict, trace: bool = False):
    """Run on 8 NeuronCores. Returns (out [8,T,D] f32, exec_time_ns|None)."""
    x = np.ascontiguousarray(np.asarray(inputs["x"], dtype=np.float32))
    wk = np.ascontiguousarray(np.asarray(inputs["Wk"], dtype=np.float32))
    wq = np.ascontiguousarray(np.asarray(inputs["Wq"], dtype=np.float32))
    wv = np.ascontiguousarray(np.asarray(inputs["Wv"], dtype=np.float32))
    assert x.shape == (N_CORES, T, E), x.shape

    nc = _get_nc()
    in_maps = [{"x": x[i], "Wk": wk, "Wq": wq, "Wv": wv}
               for i in range(N_CORES)]
    res = run_bass_kernel_spmd(nc, in_maps, core_ids=list(range(N_CORES)),
                               trace=trace)
    out = np.stack([res.results[i]["out"] for i in range(N_CORES)], axis=0)
    return out, res.exec_time_ns


def kernel(**inputs) -> np.ndarray:
    out, _ = run(inputs, trace=False)
    return out
